# revision 1
# baseline (speedup 1.0000x reference)
"""Multi-head attention (B=4, S=2048, d_model=1024, H=16) on 8 TRN2 NeuronCores.

Sharding: core c handles batch c//2 and query rows [1024*(c%2), 1024*(c%2)+1024).
Each core redundantly projects K/V for its batch (no collectives needed) and
produces a disjoint [1024, 1024] slice of the output.

Per-core pipeline:
  phase V: V = v @ w_v + b_v in row layout [t, 16*65] (col 65h+64 := 1.0 so the
           attnV matmul's 65th output row accumulates sum(exp(scores)) for free)
  phase KQ: KT/QT in channel-major layout per head pair (fp32r matmuls)
  attention (per pair, per 512-query block): scoresT = K_h @ Q_h^T with the two
           heads of a pair run concurrently on disjoint PE row groups (K=64);
           exp on ScalarE (1/sqrt(d_k) folded into the activation scale);
           attnV with M=65 accumulating over 16 key chunks; unnormalized
           outputs + softmax denominators spill to DRAM
  epilogue: one dense 128-lane reciprocal of all 16K denominators (via DRAM
           reshape bounce); K=2 ones-matmul broadcasts recips across
           partitions; normalize; out-projection (fp32r) + bias; DMA out.
"""

import numpy as np

import bass_rust
import concourse.bass as bass
import concourse.mybir as mybir
import concourse.tile as tile
from concourse.bass_utils import run_bass_kernel_spmd
from concourse.vector_clock import ScopedClock

F32 = mybir.dt.float32
F32R = mybir.dt.float32r
BF16 = mybir.dt.bfloat16
AF = mybir.ActivationFunctionType
ADD = mybir.AluOpType.add
MULT = mybir.AluOpType.mult

D_MODEL = 1024
B = 4
S = 2048
N_CORES = 8
QL = 1024  # query rows per core
NPAIR = 8  # head pairs
NK = D_MODEL // 128  # contraction chunks
NT = S // 128  # key chunks
VPW = 65 * 16  # padded V width

# ---------------------------------------------------------------------------
# Workaround for this container's walrus build: each instruction may carry at
# most ONE embedded sync-wait ("Too many sync wait commands" otherwise). Tile
# attaches several; split the extras onto same-engine NOPs placed immediately
# before the instruction (engine queues are in-order => identical semantics).
_MAX_WAITS = 1


def _patched_lower(self, ordered):
    nc = self.nc
    for bb_name, insts in ordered.items():
        new_list = []
        for inst in insts:
            si = inst.sync_info
            waits = list(si.on_wait) if si is not None and si.on_wait else []
            if len(waits) > _MAX_WAITS:
                updates = list(si.on_update) if si.on_update else []
                for w in waits[:-_MAX_WAITS]:
                    nop = bass_rust.InstNoOp(
                        name=nc.get_next_instruction_name(),
                        engine=inst.engine,
                        debug=inst.debug,
                        sync_info=bass_rust.SyncInfo(on_wait=[w], on_update=[]),
                    )
                    new_list.append(nop)
                inst.sync_info = bass_rust.SyncInfo(
                    on_wait=waits[-_MAX_WAITS:], on_update=updates
                )
            new_list.append(inst)
        insts[:] = new_list
    return tile.TileContext._orig_lower_ordered_insts(self, ordered)


def _patched_drain(self, tick_clock, wait_clock):
    probe = self.nc.sync.nop(nofuse=True)
    wait_clock.add_sem_waits(probe.ins, ScopedClock({None: tick_clock.global_clock}))
    si = probe.ins.sync_info
    waits = list(si.on_wait) if si is not None and si.on_wait else []
    if len(waits) > _MAX_WAITS:
        probe.ins.sync_info = bass_rust.SyncInfo(
            on_wait=waits[:_MAX_WAITS], on_update=[]
        )
        for w in waits[_MAX_WAITS:]:
            n = self.nc.sync.nop(nofuse=True)
            n.ins.sync_info = bass_rust.SyncInfo(on_wait=[w], on_update=[])
    self.nc.sync.drain()
    self.nc.all_engine_barrier()
    assert self.sems is not None
    popped = self.nc._tile_sem_poison_stack.pop()
    assert popped is self._sem_poison
    self.nc.clear_and_free_semaphores(list(self.sems.allocated().values()))
    self.nc.all_engine_barrier()




import concourse.bass_utils as _bu


def _bvo_ldwopt(
    tmpdir, inp="bir.json", outp="file.neff", arch=None, *, dve_root=None
):
    cmd = [
        _bu.get_walrus_driver(),
        "--pass",
        ",".join(
            [
                "birverifier",
                "runtime_memory_reservation",
                "lower_act",
                "lower_dve",
                "lower_ap_offset",
                "codegen",
                "neff_packager",
            ]
        ),
        "-i",
        inp,
        "--neff-output-filename",
        outp,
        "--enable-birsim=true",
        "--mem-mode=physical",
        "--policy=0",
        "--enable-ldw-opt=true",
        "--assign-static-dmas-to-sp=false",
        f"--dram-page-size={_bu.aot_getenv('NEURON_SCRATCHPAD_PAGE_SIZE', '256')}",
        "--enable-neff-debug-info=true",
        "--jobs",
        "8",
        *_bu.get_walrus_args(
            _bu.get_bir_arch(tmpdir, inp) if arch is None else arch,
            tmpdir,
            dve_root=dve_root,
        ),
    ]
    result = _bu.run_command(cmd, cwd=tmpdir)
    if result is not None:
        (_bu.Path(tmpdir) / "log.txt").write_text(result.stdout)
    return f"{tmpdir}/{outp}"


def _install_ldwopt():
    import os

    # ldw-opt splits fp32r matmuls into standalone InstLdweights, which this
    # walrus cannot codegen (the reason the flag ships disabled). Opt-in only.
    if os.environ.get("USE_LDW_OPT") == "1":
        _bu.bir_verify_and_optimise = _bvo_ldwopt


def _install_patch():
    _install_ldwopt()
    if not hasattr(tile.TileContext, "_orig_lower_ordered_insts"):
        tile.TileContext._orig_lower_ordered_insts = (
            tile.TileContext._lower_ordered_insts
        )
        tile.TileContext._lower_ordered_insts = _patched_lower
        tile.TileContext._drain_and_barrier = _patched_drain


# ---------------------------------------------------------------------------


def _build_bass():
    nc = bass.Bass()
    qt = nc.dram_tensor("qt", [D_MODEL, QL], BF16, kind="ExternalInput")
    kt = nc.dram_tensor("kt", [D_MODEL, S], BF16, kind="ExternalInput")
    vt = nc.dram_tensor("vt", [NT, 128, 1024], BF16, kind="ExternalInput")
    wq = nc.dram_tensor("wq", [NPAIR, NK, 128, 128], BF16, kind="ExternalInput")
    wk = nc.dram_tensor("wk", [NPAIR, NK, 128, 128], BF16, kind="ExternalInput")
    wv = nc.dram_tensor("wv", [D_MODEL, D_MODEL], BF16, kind="ExternalInput")
    wo = nc.dram_tensor("wo", [D_MODEL, D_MODEL], BF16, kind="ExternalInput")
    bqt = nc.dram_tensor("bqt", [128, NK], F32, kind="ExternalInput")
    bkt = nc.dram_tensor("bkt", [128, NK], F32, kind="ExternalInput")
    bvr = nc.dram_tensor("bvr", [128, D_MODEL], F32, kind="ExternalInput")
    bor = nc.dram_tensor("bor", [128, D_MODEL], F32, kind="ExternalInput")
    ones2 = nc.dram_tensor("ones2", [128, 128], F32R, kind="ExternalInput")
    vones = nc.dram_tensor("vones", [128, 16], BF16, kind="ExternalInput")
    out = nc.dram_tensor("out", [QL, D_MODEL], F32, kind="ExternalOutput")
    xau = nc.dram_tensor("xau", [D_MODEL, QL], F32)  # unnormalized X_attn^T
    sums_d = nc.dram_tensor("sums_d", [128, 128], F32)
    sums_r = nc.dram_tensor("sums_r", [128, 128], F32)

    with tile.TileContext(nc) as tc:
        _emit(nc, tc, locals())
    return nc


def _emit(nc, tc, t):
    qt, kt, vt = t["qt"], t["kt"], t["vt"]
    wq, wk, wv, wo = t["wq"], t["wk"], t["wv"], t["wo"]
    bqt, bkt, bvr, bor = t["bqt"], t["bkt"], t["bvr"], t["bor"]
    ones2, out, vones = t["ones2"], t["out"], t["vones"]
    xau, sums_d, sums_r = t["xau"], t["sums_d"], t["sums_r"]

    P = tc.tile_pool

    with (
        P(name="consts", bufs=1) as consts,
        P(name="stg", bufs=2) as stg,
    ):
        ones_t = consts.tile([128, 128], F32R, tag="ones2")
        nc.sync.dma_start(ones_t[:], ones2[:])
        bqt_t = consts.tile([128, NK], F32, tag="bqt")
        nc.sync.dma_start(bqt_t[:], bqt[:])
        bkt_t = consts.tile([128, NK], F32, tag="bkt")
        nc.sync.dma_start(bkt_t[:], bkt[:])
        bvr_t = consts.tile([128, D_MODEL], F32, tag="bvr")
        nc.sync.dma_start(bvr_t[:], bvr[:])
        bor_t = consts.tile([128, D_MODEL], F32, tag="bor")
        nc.sync.dma_start(bor_t[:], bor[:])

        with P(name="pv", bufs=1) as pv, P(name="pkq", bufs=1) as pkq:
            # ---- V projection (bf16): V_pad [t, 16*65] row-major ---------
            v_tiles = []
            for c in range(NT):
                v = pv.tile([128, VPW], BF16, name=f"v{c}", tag=f"v{c}")
                v_tiles.append(v)

            with (
                P(name="wvp", bufs=1) as wvp,
                P(name="vstr", bufs=5) as vstr,
                P(name="psV", bufs=3, space="PSUM") as psV,
            ):
                wv_tiles = []
                for k in range(NK):
                    wvt = wvp.tile([128, D_MODEL], BF16, name=f"wv{k}", tag=f"wv{k}")
                    nc.sync.dma_start(wvt[:], wv[128 * k : 128 * k + 128, :])
                    wv_tiles.append(wvt)
                for c in range(NT):
                    vts = vstr.tile([128, 1024], BF16, tag="vts")
                    nc.sync.dma_start(vts[:], vt[c])
                    ps = psV.tile([128, 1024], F32, tag="vproj")
                    for k in range(NK):
                        for j in range(2):
                            nc.tensor.matmul(
                                ps[:, 512 * j : 512 * j + 512],
                                vts[:, 128 * k : 128 * k + 128],
                                wv_tiles[k][:, 512 * j : 512 * j + 512],
                                start=(k == 0),
                                stop=(k == NK - 1),
                                skip_group_check=True,
                            )
                    dst = v_tiles[c][:, :].rearrange("p (h w) -> p h w", w=65)[
                        :, :, 0:64
                    ]
                    nc.vector.tensor_tensor(
                        dst,
                        ps[:, :].rearrange("p (h w) -> p h w", w=64),
                        bvr_t[:, :].rearrange("p (h w) -> p h w", w=64),
                        ADD,
                    )

            for c in range(NT):
                nc.gpsimd.dma_start(
                    v_tiles[c][:, :].rearrange("p (h w) -> p h w", w=65)[:, :, 64:65],
                    vones[:, :, None],
                )

            # ---- K/Q projections (bf16) interleaved into attention -------
            sums_flat = sums_d[:, :].rearrange("p f -> (p f)")
            sums_r_flat = sums_r[:, :].rearrange("p f -> (p f)")
            xn_tiles = [None] * NPAIR
            KT = [
                pkq.tile([128, S], BF16, name=f"ktg{g}", tag=f"ktg{g}")
                for g in range(NPAIR)
            ]
            QT = [
                pkq.tile([128, QL], BF16, name=f"qtg{g}", tag=f"qtg{g}")
                for g in range(NPAIR)
            ]
            pxn = None  # set below; must outlive into the out-projection

            with (
                P(name="kstr", bufs=1) as kstr,
                P(name="qstr", bufs=1) as qstr,
                P(name="wks", bufs=2) as wks,
                P(name="wqs", bufs=2) as wqs,
                P(name="expp", bufs=6) as expp,
                P(name="psS", bufs=2, space="PSUM") as psS,
                P(name="psacc", bufs=1, space="PSUM") as psacc,
                P(name="psP", bufs=1, space="PSUM") as psP,
                P(name="ph3s", bufs=2) as ph3s,
            ):
                pxn = pkq  # xn tiles live in the long-lived pkq pool
                kfull = []
                for k in range(NK):
                    ktile = kstr.tile([128, S], BF16, name=f"ktf{k}", tag=f"ktf{k}")
                    nc.sync.dma_start(ktile[:], kt[128 * k : 128 * k + 128, :])
                    kfull.append(ktile)
                qfull = []
                for k in range(NK):
                    qtile = qstr.tile([128, QL], BF16, name=f"qtf{k}", tag=f"qtf{k}")
                    nc.sync.dma_start(qtile[:], qt[128 * k : 128 * k + 128, :])
                    qfull.append(qtile)

                def emit_kproj(half, g):
                    wkg = []
                    for k in range(NK):
                        wkt = wks.tile([128, 128], BF16, tag=f"wks{k}")
                        nc.sync.dma_start(wkt[:], wk[g, k])
                        wkg.append(wkt)
                    ps = psP.tile([128, 1024], F32, tag="kproj")
                    for k in range(NK):
                        for j in range(2):
                            nc.tensor.matmul(
                                ps[:, 512 * j : 512 * j + 512],
                                wkg[k][:],
                                kfull[k][
                                    :,
                                    1024 * half + 512 * j : 1024 * half + 512 * j + 512,
                                ],
                                start=(k == 0),
                                stop=(k == NK - 1),
                                skip_group_check=True,
                            )
                    nc.vector.tensor_scalar_add(
                        KT[g][:, 1024 * half : 1024 * half + 1024],
                        ps[:],
                        bkt_t[:, g : g + 1],
                    )

                def emit_qproj(g):
                    wqg = []
                    for k in range(NK):
                        wqt = wqs.tile([128, 128], BF16, tag=f"wqs{k}")
                        nc.sync.dma_start(wqt[:], wq[g, k])
                        wqg.append(wqt)
                    ps = psP.tile([128, 1024], F32, tag="kproj")
                    for k in range(NK):
                        for j in range(2):
                            nc.tensor.matmul(
                                ps[:, 512 * j : 512 * j + 512],
                                wqg[k][:],
                                qfull[k][:, 512 * j : 512 * j + 512],
                                start=(k == 0),
                                stop=(k == NK - 1),
                                skip_group_check=True,
                            )
                    nc.vector.tensor_scalar_add(QT[g][:], ps[:], bqt_t[:, g : g + 1])

                for g in range(3):
                    emit_kproj(0, g)
                    emit_kproj(1, g)
                    emit_qproj(g)

                proj_steps = []
                for g in range(3, NPAIR):
                    proj_steps.append((emit_kproj, (0, g)))
                    proj_steps.append((emit_kproj, (1, g)))
                    proj_steps.append((emit_qproj, (g,)))
                proj_iter = iter(proj_steps)

                def emit_scores(g, qb, cg):
                    ktg, qtg = KT[g], QT[g]
                    q0 = 512 * qb
                    tiles = []
                    for h in range(2):
                        p0 = 64 * h
                        sc = psS.tile([128, QL], F32, tag="scores")
                        for ci in range(2):
                            c = 2 * cg + ci
                            nc.tensor.matmul(
                                sc[:, 512 * ci : 512 * ci + 512],
                                ktg[p0 : p0 + 64, 128 * c : 128 * c + 128],
                                qtg[p0 : p0 + 64, q0 : q0 + 512],
                                start=True,
                                stop=True,
                                skip_group_check=True,
                            )
                        tiles.append(sc)
                    return tiles

                def emit_spill(g, qb, acc):
                    q0 = 512 * qb
                    for h in range(2):
                        sg = stg.tile([65, 512], F32, tag="spill")
                        nc.vector.tensor_copy(sg[:], acc[h][0:65, :])
                        nc.sync.dma_start(
                            xau[
                                128 * g + 64 * h : 128 * g + 64 * h + 64,
                                q0 : q0 + 512,
                            ],
                            sg[0:64, :],
                        )
                        base = g * 2048 + h * 1024 + 512 * qb
                        nc.sync.dma_start(
                            sums_flat[base : base + 512][None, :], sg[64:65, :]
                        )

                def emit_recip_batch(glo, ghi):
                    r0, r1 = 16 * glo, 16 * ghi
                    den = stg.tile([16 * (ghi - glo), 128], F32, tag="dense")
                    nc.sync.dma_start(den[:], sums_d[r0:r1, :])
                    denr = stg.tile([16 * (ghi - glo), 128], F32, tag="denser")
                    nc.vector.reciprocal(denr[:], den[:])
                    nc.sync.dma_start(sums_r[r0:r1, :], denr[:])

                def emit_chain(g):
                    srr = ph3s.tile([128, QL], F32R, tag="srr")
                    for h in range(2):
                        base = g * 2048 + h * 1024
                        nc.gpsimd.dma_start(
                            srr[64 + h : 65 + h, :],
                            sums_r_flat[base : base + QL][None, :],
                        )
                    xr = ph3s.tile([128, QL], F32, tag="xr")
                    nc.sync.dma_start(xr[:], xau[128 * g : 128 * g + 128, :])
                    rep = psP.tile([128, QL], F32, tag="kproj")
                    for j in range(2):
                        nc.tensor.matmul(
                            rep[:, 512 * j : 512 * j + 512],
                            ones_t[64:66, :],
                            srr[64:66, 512 * j : 512 * j + 512],
                            start=True,
                            stop=True,
                            skip_group_check=True,
                        )
                    xn = pxn.tile([128, QL], BF16, name=f"xn{g}", tag=f"xn{g}")
                    nc.vector.tensor_tensor(xn[:], xr[:], rep[:], MULT)
                    xn_tiles[g] = xn

                chain_iter = iter(range(6))
                pending_spill = None
                slot = 0
                for g in range(NPAIR):
                    for qb in range(2):
                        acc = [
                            psacc.tile([65, 512], F32, name="acca", tag="acca"),
                            psacc.tile([65, 512], F32, name="accb", tag="accb"),
                        ]
                        sc_cur = emit_scores(g, qb, 0)
                        if pending_spill is not None:
                            emit_spill(*pending_spill)
                            pending_spill = None
                            if g == 6 and qb == 0:
                                emit_recip_batch(0, 6)
                        for cg in range(NT // 2):
                            sc_next = (
                                emit_scores(g, qb, cg + 1)
                                if cg + 1 < NT // 2
                                else None
                            )
                            for h in range(2):
                                hh = 2 * g + h
                                ex = expp.tile([128, QL], BF16, tag="exp")
                                nc.scalar.activation(
                                    ex[:], sc_cur[h][:], AF.Exp, scale=0.125
                                )
                                for ci in range(2):
                                    c = 2 * cg + ci
                                    nc.tensor.matmul(
                                        acc[h][:],
                                        v_tiles[c][:, 65 * hh : 65 * hh + 65],
                                        ex[:, 512 * ci : 512 * ci + 512],
                                        start=(c == 0),
                                        stop=(c == NT - 1),
                                        skip_group_check=True,
                                    )
                            if g >= 2 and slot % 3 == 0:
                                step = next(proj_iter, None)
                                if step is not None:
                                    step[0](*step[1])
                            if 2 * g + qb >= 13 and slot % 4 == 0:
                                cidx = next(chain_iter, None)
                                if cidx is not None:
                                    emit_chain(cidx)
                            if g >= 2:
                                slot += 1
                            sc_cur = sc_next
                        pending_spill = (g, qb, acc)
                emit_spill(*pending_spill)
                for step in proj_iter:
                    step[0](*step[1])
                emit_recip_batch(6, NPAIR)
                for cidx in chain_iter:
                    emit_chain(cidx)
                emit_chain(6)
                emit_chain(7)

        # ---- output projection (bf16) ------------------------------------
        with (
            P(name="pwo", bufs=1) as pwo,
            P(name="ps3o", bufs=2, space="PSUM") as ps3o,
        ):
            wo_tiles = []
            for k in range(NK):
                wot = pwo.tile([128, D_MODEL], BF16, name=f"wo{k}", tag=f"wo{k}")
                nc.sync.dma_start(wot[:], wo[128 * k : 128 * k + 128, :])
                wo_tiles.append(wot)

            for m in range(QL // 128):
                ps = ps3o.tile([128, D_MODEL], F32, tag="oproj")
                for g in range(NPAIR):
                    for j in range(2):
                        nc.tensor.matmul(
                            ps[:, 512 * j : 512 * j + 512],
                            xn_tiles[g][:, 128 * m : 128 * m + 128],
                            wo_tiles[g][:, 512 * j : 512 * j + 512],
                            start=(g == 0),
                            stop=(g == NPAIR - 1),
                            skip_group_check=True,
                        )
                ot = stg.tile([128, D_MODEL], F32, tag="outs")
                nc.vector.tensor_tensor(ot[:], ps[:], bor_t[:], ADD)
                nc.sync.dma_start(out[128 * m : 128 * m + 128, :], ot[:])


_NC_CACHE = None
LAST_RESULT = None


def _get_nc():
    global _NC_CACHE
    if _NC_CACHE is None:
        _install_patch()
        _NC_CACHE = _build_bass()
    return _NC_CACHE


def kernel(q, k, v, w_q, b_q, w_k, b_k, w_v, b_v, w_o, b_o):
    global LAST_RESULT
    import ml_dtypes

    q = np.asarray(q, np.float32)
    k = np.asarray(k, np.float32)
    v = np.asarray(v, np.float32)
    def _tile_w(w):
        # [in, out] -> [g, k, 128, 128] contiguous tiles: w[128k:+128, 128g:+128]
        return np.ascontiguousarray(
            np.asarray(w, np.float32)
            .reshape(NK, 128, NPAIR, 128)
            .transpose(2, 0, 1, 3)
        ).astype(ml_dtypes.bfloat16)

    w_q = _tile_w(w_q)
    w_k = _tile_w(w_k)
    w_v = np.asarray(w_v, np.float32).astype(ml_dtypes.bfloat16)
    w_o = np.asarray(w_o, np.float32).astype(ml_dtypes.bfloat16)
    b_q = np.asarray(b_q, np.float32)
    b_k = np.asarray(b_k, np.float32)
    b_v = np.asarray(b_v, np.float32)
    b_o = np.asarray(b_o, np.float32)

    bqt = np.ascontiguousarray(b_q.reshape(NK, 128).T)
    bkt = np.ascontiguousarray(b_k.reshape(NK, 128).T)
    bvr = np.ascontiguousarray(np.broadcast_to(b_v[None, :], (128, D_MODEL)))
    bor = np.ascontiguousarray(np.broadcast_to(b_o[None, :], (128, D_MODEL)))
    ones2 = np.zeros((128, 128), np.float32)
    ones2[64, 0:64] = 1.0
    ones2[65, 64:128] = 1.0
    vones_np = np.ones((128, 16), ml_dtypes.bfloat16)

    in_maps = []
    for c in range(N_CORES):
        b = c // 2
        r0 = QL * (c % 2)
        in_maps.append(
            {
                "qt": np.ascontiguousarray(q[b, r0 : r0 + QL, :].T).astype(
                    ml_dtypes.bfloat16
                ),
                "kt": np.ascontiguousarray(k[b].T).astype(ml_dtypes.bfloat16),
                "vt": np.ascontiguousarray(
                    v[b]
                    .T.reshape(8, 128, 16, 128)
                    .transpose(2, 1, 0, 3)
                    .reshape(16, 128, 1024)
                ).astype(ml_dtypes.bfloat16),
                "wq": w_q,
                "wk": w_k,
                "wv": w_v,
                "wo": w_o,
                "bqt": bqt,
                "bkt": bkt,
                "bvr": bvr,
                "bor": bor,
                "ones2": ones2,
                "vones": vones_np,
            }
        )

    nc = _get_nc()
    res = run_bass_kernel_spmd(nc, in_maps, list(range(N_CORES)))
    LAST_RESULT = res

    outp = np.empty((B, S, D_MODEL), np.float32)
    for c in range(N_CORES):
        b = c // 2
        r0 = QL * (c % 2)
        outp[b, r0 : r0 + QL, :] = res.results[c]["out"]
    return outp

